# revision 25
# baseline (speedup 1.0000x reference)
"""Trainium2 Bass kernel for nn_FragAttention (segment_reduce).

Reference computation (S=128, B=512, D=512, G=S-1=127):
    xb     = transpose(x, (1,0,2))            # (B, S, D)
    xm     = xb * (~src_mask)[:, :, None]     # zero padded tokens
    left   [b,g,d] = sum_{s<=g} xm[b,s,d]     # masked prefix sums
    right  [b,g,d] = sum_{s>g}  xm[b,s,d]
    out    = concat([left, right], axis=2)    # (B, G, 2D)

Strategy: pure data parallel over B across 8 cores (64 batches each).
The pad mask is folded into x on the host (exact: multiply by 0/1), and
x is cast to bf16 on the host — halves input HBM traffic; the 0/1
triangular weights are exact in bf16 so only x's mantissa truncation
matters (~2e-3 rel err, gate is 2e-2). Per batch the prefix/suffix sums
are computed on the TensorEngine as two matmuls against constant 0/1
triangular matrices (contraction over S=128 on partitions, f32 PSUM
accumulate), then PSUM->SBUF copies (DVE for even batches, ACT for odd)
cast the result to bf16. The host upcasts the returned bf16 block.

DMA (the roofline resource: ~25 MB/core at ~358 GB/s HBM): reads use
HWDGE (scalar/ACT ring), which sprays each transfer's per-partition
descriptors across all 16 SDMA engines by destination SBUF port
(~300 GB/s measured); all 4 input chunks are issued eagerly up front so
they stop contending with writes early. HWDGE does NOT spray
HBM-destined writes - all descriptors land on one engine
(~27 GB/s/ring, measured) - so most output writes go through SWDGE
(gpsimd): each DMA binds to one engine (~20 GB/s effective incl. the
per-512B 4-byte completion writes) and Tile's 8 DMASW sem lanes keep 8
in flight (~160 GB/s). On top of that, a few chunks are routed to the
otherwise-idle sync(SP)/scalar(ACT) HWDGE rings (~27 GB/s each).
Output is written g-major (out[g, b, 2D]) so one partition row is a
128 KB contiguous DRAM run: OUT_CHUNK=2 batches -> 127 descriptors of
4 KB per DMA.
"""

import numpy as np
import ml_dtypes

import concourse.bass as bass
import concourse.mybir as mybir
from concourse import bacc
from concourse.tile import TileContext
from concourse.bass_utils import run_bass_kernel_spmd

S, B, D = 128, 512, 512
G = S - 1
N_CORES = 8
BL = B // N_CORES  # 64 batches per core

IN_CHUNK = 16  # batches per input DMA  (16 KB per-partition descriptors)
OUT_CHUNK = 4  # batches per output DMA (8 KB per-partition descriptors)

_NC_CACHE = None


def _build_bass() -> bass.Bass:
    nc = bacc.Bacc()
    f32 = mybir.dt.float32
    bf16 = mybir.dt.bfloat16

    x_in = nc.declare_dram_parameter("x", [S, BL, D], bf16, isOutput=False)
    # tri[:, 0:128] = upper (incl diag)  tri[s,g] = 1 if s <= g  -> prefix sums
    # tri[:, 128:256] = strictly lower   tri[s,g] = 1 if s >  g  -> suffix sums
    t_in = nc.declare_dram_parameter("tri", [S, 2 * S], bf16, isOutput=False)
    # g-major per-core output: partition row g maps to a contiguous DRAM run,
    # host transposes (G, BL, 2D) -> (BL, G, 2D) while gathering.
    # Padded to S=128 rows: a 128-partition SBUF side lets the DGE spray the
    # transfer's descriptors across all 16 SDMA engines (8 partitions per
    # engine); a 127-partition AP falls off the swizzle path and the whole
    # DMA binds to ONE engine (~23 GB/s). Row 127 is garbage; host drops it.
    out = nc.declare_dram_parameter("out", [S, BL, 2 * D], bf16, isOutput=True)

    with TileContext(nc) as tc:
        with (
            tc.tile_pool(name="const", bufs=1) as cpool,
            tc.tile_pool(name="xin", bufs=5) as xpool,
            tc.tile_pool(name="outs", bufs=8) as opool,
            tc.tile_pool(name="psum", bufs=2, space="PSUM") as ppool,
        ):
            tri = cpool.tile([S, 2 * S], bf16)
            nc.sync.dma_start(out=tri[:], in_=t_in[:])
            ut = tri[:, 0:S]        # (128, 128) stationary, left sums
            lt = tri[:, S : 2 * S]  # (128, 128) stationary, right sums

            def per_pair(xt, ot, j, k, use_dve):
                """2 batches (j, j+1) of xt -> slots (k, k+1) of ot.

                One 4-bank PSUM tile takes all 4 matmuls (a matmul's output
                cannot exceed one 512-f32 PSUM bank per partition, so
                batches cannot share a wider matmul), then ONE 4-bank copy
                into ot. The copy engine alternates per ot tile (DVE for
                even chunks, ACT for odd): Tile tracks ot writes at tile
                granularity, so two engines writing disjoint slices of the
                same tile serialize on a false dependency - one tile, one
                engine keeps both copy engines fully parallel across chunks.
                """
                ps = ppool.tile([S, 4, D], f32)  # 4 adjacent banks
                for h, (b, tri_) in enumerate(
                    [(j, ut), (j, lt), (j + 1, ut), (j + 1, lt)]
                ):
                    nc.tensor.matmul(out=ps[:, h, :], lhsT=tri_, rhs=xt[:, b, :],
                                     start=True, stop=True)
                dst = ot[:, k : k + 2, :].rearrange(
                    "g c (h d) -> g (c h) d", h=2
                )
                if use_dve:
                    nc.vector.tensor_copy(out=dst, in_=ps[:, :, :])
                else:
                    nc.scalar.activation(
                        out=dst, in_=ps[:, :, :],
                        func=mybir.ActivationFunctionType.Copy,
                    )

            # issue all input loads eagerly (ACT HWDGE ring, sprays all 16
            # engines) so read traffic is done before writes ramp up
            # (finer-grained read plans tested worse: a 5-read split drags
            # the read tail to ~39us and slows the write drain)
            READS = [(0, 16), (16, 16), (32, 16), (48, 16)]
            xts = {}  # batch index of chunk start -> (tile, base batch)
            for r0, rn in READS:
                xt = xpool.tile([S, rn, D], bf16)
                nc.scalar.dma_start(out=xt[:], in_=x_in[:, r0 : r0 + rn, :])
                for b in range(r0, r0 + rn, OUT_CHUNK):
                    xts[b] = (xt, r0)

            n_chunks = BL // OUT_CHUNK
            for ci in range(n_chunks):
                o0 = ci * OUT_CHUNK
                xt, xbase = xts[o0]
                ot = opool.tile([S, OUT_CHUNK, 2 * D], bf16)
                for j in range(0, OUT_CHUNK, 2):
                    per_pair(xt, ot, o0 - xbase + j, j, ci % 2 == 0)
                nc.gpsimd.dma_start(
                    out=out[:, o0 : o0 + OUT_CHUNK, :], in_=ot[:, :, :],
                )
    nc.finalize()  # runs the Bacc pass pipeline (reg alloc, wait splitting)
    return nc


def _get_nc() -> bass.Bass:
    global _NC_CACHE
    if _NC_CACHE is None:
        _NC_CACHE = _build_bass()
    return _NC_CACHE


def _make_in_maps(x: np.ndarray, src_mask: np.ndarray) -> list[dict]:
    x = np.asarray(x, dtype=np.float32)
    src_mask = np.asarray(src_mask)
    assert x.shape == (S, B, D), x.shape
    assert src_mask.shape == (B, S), src_mask.shape

    valid = (~src_mask.astype(bool)).astype(np.float32).T  # (S, B)
    xm = (x * valid[:, :, None]).astype(ml_dtypes.bfloat16)
    tri = np.concatenate(
        [
            np.triu(np.ones((S, S), np.float32)),       # s <= g
            np.tril(np.ones((S, S), np.float32), -1),   # s >  g
        ],
        axis=1,
    ).astype(ml_dtypes.bfloat16)

    in_maps = []
    for i in range(N_CORES):
        sl = slice(i * BL, (i + 1) * BL)
        in_maps.append(
            {
                "x": np.ascontiguousarray(xm[:, sl, :]),
                "tri": tri,
            }
        )
    return in_maps


def _assemble(results: list[dict]) -> np.ndarray:
    full = np.empty((B, G, 2 * D), dtype=np.float32)
    for i in range(N_CORES):
        full[i * BL : (i + 1) * BL] = (
            results[i]["out"][:G].transpose(1, 0, 2).astype(np.float32)
        )
    return full


def kernel(x: np.ndarray, src_mask: np.ndarray) -> np.ndarray:
    in_maps = _make_in_maps(x, src_mask)
    res = run_bass_kernel_spmd(_get_nc(), in_maps, core_ids=list(range(N_CORES)))
    return _assemble(res.results)


# revision 26
# speedup vs baseline: 1.0218x; 1.0218x over previous
"""Trainium2 Bass kernel for nn_FragAttention (segment_reduce).

Reference computation (S=128, B=512, D=512, G=S-1=127):
    xb     = transpose(x, (1,0,2))            # (B, S, D)
    xm     = xb * (~src_mask)[:, :, None]     # zero padded tokens
    left   [b,g,d] = sum_{s<=g} xm[b,s,d]     # masked prefix sums
    right  [b,g,d] = sum_{s>g}  xm[b,s,d]
    out    = concat([left, right], axis=2)    # (B, G, 2D)

Strategy: pure data parallel over B across 8 cores (64 batches each).
The pad mask is folded into x on the host (exact: multiply by 0/1), and
x is cast to bf16 on the host — halves input HBM traffic; the 0/1
triangular weights are exact in bf16 so only x's mantissa truncation
matters (~2e-3 rel err, gate is 2e-2). Per batch the prefix/suffix sums
are computed on the TensorEngine as two matmuls against constant 0/1
triangular matrices (contraction over S=128 on partitions, f32 PSUM
accumulate), then PSUM->SBUF copies (DVE for even batches, ACT for odd)
cast the result to bf16. The host upcasts the returned bf16 block.

DMA (the roofline resource: ~25 MB/core at ~358 GB/s HBM): reads use
HWDGE (scalar/ACT ring), which sprays each transfer's per-partition
descriptors across all 16 SDMA engines by destination SBUF port
(~300 GB/s measured); all 4 input chunks are issued eagerly up front so
they stop contending with writes early. HWDGE does NOT spray
HBM-destined writes - all descriptors land on one engine
(~27 GB/s/ring, measured) - so most output writes go through SWDGE
(gpsimd): each DMA binds to one engine (~20 GB/s effective incl. the
per-512B 4-byte completion writes) and Tile's 8 DMASW sem lanes keep 8
in flight (~160 GB/s). On top of that, a few chunks are routed to the
otherwise-idle sync(SP)/scalar(ACT) HWDGE rings (~27 GB/s each).
Output is written g-major (out[g, b, 2D]) so one partition row is a
128 KB contiguous DRAM run: OUT_CHUNK=2 batches -> 127 descriptors of
4 KB per DMA.
"""

import numpy as np
import ml_dtypes

import concourse.bass as bass
import concourse.mybir as mybir
from concourse import bacc
from concourse.tile import TileContext
from concourse.bass_utils import run_bass_kernel_spmd

S, B, D = 128, 512, 512
G = S - 1
N_CORES = 8
BL = B // N_CORES  # 64 batches per core

IN_CHUNK = 16  # batches per input DMA  (16 KB per-partition descriptors)
OUT_CHUNK = 4  # batches per output DMA (8 KB per-partition descriptors)

_NC_CACHE = None


def _build_bass() -> bass.Bass:
    nc = bacc.Bacc()
    f32 = mybir.dt.float32
    bf16 = mybir.dt.bfloat16

    x_in = nc.declare_dram_parameter("x", [S, BL, D], bf16, isOutput=False)
    # tri[:, 0:128] = upper (incl diag)  tri[s,g] = 1 if s <= g  -> prefix sums
    # tri[:, 128:256] = strictly lower   tri[s,g] = 1 if s >  g  -> suffix sums
    t_in = nc.declare_dram_parameter("tri", [S, 2 * S], bf16, isOutput=False)
    # g-major per-core output: partition row g maps to a contiguous DRAM run,
    # host transposes (G, BL, 2D) -> (BL, G, 2D) while gathering.
    # Padded to S=128 rows: a 128-partition SBUF side lets the DGE spray the
    # transfer's descriptors across all 16 SDMA engines (8 partitions per
    # engine); a 127-partition AP falls off the swizzle path and the whole
    # DMA binds to ONE engine (~23 GB/s). Row 127 is garbage; host drops it.
    out = nc.declare_dram_parameter("out", [S, BL, 2 * D], bf16, isOutput=True)

    with TileContext(nc) as tc:
        with (
            tc.tile_pool(name="const", bufs=1) as cpool,
            tc.tile_pool(name="xin", bufs=5) as xpool,
            tc.tile_pool(name="outs", bufs=8) as opool,
            tc.tile_pool(name="psum", bufs=2, space="PSUM") as ppool,
        ):
            tri = cpool.tile([S, 2 * S], bf16)
            nc.sync.dma_start(out=tri[:], in_=t_in[:])
            ut = tri[:, 0:S]        # (128, 128) stationary, left sums
            lt = tri[:, S : 2 * S]  # (128, 128) stationary, right sums

            def per_pair(xt, ot, j, k, use_dve):
                """2 batches (j, j+1) of xt -> slots (k, k+1) of ot.

                One 4-bank PSUM tile takes all 4 matmuls (a matmul's output
                cannot exceed one 512-f32 PSUM bank per partition, so
                batches cannot share a wider matmul), then ONE 4-bank copy
                into ot. The copy engine alternates per ot tile (DVE for
                even chunks, ACT for odd): Tile tracks ot writes at tile
                granularity, so two engines writing disjoint slices of the
                same tile serialize on a false dependency - one tile, one
                engine keeps both copy engines fully parallel across chunks.
                """
                ps = ppool.tile([S, 4, D], f32)  # 4 adjacent banks
                for h, (b, tri_) in enumerate(
                    [(j, ut), (j, lt), (j + 1, ut), (j + 1, lt)]
                ):
                    nc.tensor.matmul(out=ps[:, h, :], lhsT=tri_, rhs=xt[:, b, :],
                                     start=True, stop=True)
                dst = ot[:, k : k + 2, :].rearrange(
                    "g c (h d) -> g (c h) d", h=2
                )
                if use_dve:
                    nc.vector.tensor_copy(out=dst, in_=ps[:, :, :])
                else:
                    nc.scalar.activation(
                        out=dst, in_=ps[:, :, :],
                        func=mybir.ActivationFunctionType.Copy,
                    )

            # issue all input loads eagerly so read traffic is done before
            # writes ramp up. A small first read rides the otherwise-idle
            # sync(SP) HWDGE ring so chunk-0 compute starts ~5us sooner; the
            # rest go on the scalar(ACT) ring. (Splitting the ACT ring's own
            # first read instead tested worse: it drags the ACT read tail to
            # ~39us and slows the write drain.)
            READS = [(0, 8, nc.sync), (8, 8, nc.scalar), (16, 16, nc.scalar),
                     (32, 16, nc.scalar), (48, 16, nc.scalar)]
            xts = {}  # batch index of chunk start -> (tile, base batch)
            for r0, rn, eng in READS:
                xt = xpool.tile([S, rn, D], bf16)
                eng.dma_start(out=xt[:], in_=x_in[:, r0 : r0 + rn, :])
                for b in range(r0, r0 + rn, OUT_CHUNK):
                    xts[b] = (xt, r0)

            n_chunks = BL // OUT_CHUNK
            for ci in range(n_chunks):
                o0 = ci * OUT_CHUNK
                xt, xbase = xts[o0]
                ot = opool.tile([S, OUT_CHUNK, 2 * D], bf16)
                for j in range(0, OUT_CHUNK, 2):
                    per_pair(xt, ot, o0 - xbase + j, j, ci % 2 == 0)
                nc.gpsimd.dma_start(
                    out=out[:, o0 : o0 + OUT_CHUNK, :], in_=ot[:, :, :],
                )
    nc.finalize()  # runs the Bacc pass pipeline (reg alloc, wait splitting)
    return nc


def _get_nc() -> bass.Bass:
    global _NC_CACHE
    if _NC_CACHE is None:
        _NC_CACHE = _build_bass()
    return _NC_CACHE


def _make_in_maps(x: np.ndarray, src_mask: np.ndarray) -> list[dict]:
    x = np.asarray(x, dtype=np.float32)
    src_mask = np.asarray(src_mask)
    assert x.shape == (S, B, D), x.shape
    assert src_mask.shape == (B, S), src_mask.shape

    valid = (~src_mask.astype(bool)).astype(np.float32).T  # (S, B)
    xm = (x * valid[:, :, None]).astype(ml_dtypes.bfloat16)
    tri = np.concatenate(
        [
            np.triu(np.ones((S, S), np.float32)),       # s <= g
            np.tril(np.ones((S, S), np.float32), -1),   # s >  g
        ],
        axis=1,
    ).astype(ml_dtypes.bfloat16)

    in_maps = []
    for i in range(N_CORES):
        sl = slice(i * BL, (i + 1) * BL)
        in_maps.append(
            {
                "x": np.ascontiguousarray(xm[:, sl, :]),
                "tri": tri,
            }
        )
    return in_maps


def _assemble(results: list[dict]) -> np.ndarray:
    full = np.empty((B, G, 2 * D), dtype=np.float32)
    for i in range(N_CORES):
        full[i * BL : (i + 1) * BL] = (
            results[i]["out"][:G].transpose(1, 0, 2).astype(np.float32)
        )
    return full


def kernel(x: np.ndarray, src_mask: np.ndarray) -> np.ndarray:
    in_maps = _make_in_maps(x, src_mask)
    res = run_bass_kernel_spmd(_get_nc(), in_maps, core_ids=list(range(N_CORES)))
    return _assemble(res.results)


# revision 28
# speedup vs baseline: 1.1323x; 1.1082x over previous
"""Trainium2 Bass kernel for nn_FragAttention (segment_reduce).

Reference computation (S=128, B=512, D=512, G=S-1=127):
    xb     = transpose(x, (1,0,2))            # (B, S, D)
    xm     = xb * (~src_mask)[:, :, None]     # zero padded tokens
    left   [b,g,d] = sum_{s<=g} xm[b,s,d]     # masked prefix sums
    right  [b,g,d] = sum_{s>g}  xm[b,s,d]
    out    = concat([left, right], axis=2)    # (B, G, 2D)

Strategy: pure data parallel over B across 8 cores (64 batches each).
The pad mask is folded into x on the host (exact: multiply by 0/1), and
x is cast to bf16 on the host — halves input HBM traffic; the 0/1
triangular weights are exact in bf16 so only x's mantissa truncation
matters (~2e-3 rel err, gate is 2e-2). Per batch the prefix/suffix sums
are computed on the TensorEngine as two matmuls against constant 0/1
triangular matrices (contraction over S=128 on partitions, f32 PSUM
accumulate), then PSUM->SBUF copies (DVE for even batches, ACT for odd)
cast the result to bf16. The host upcasts the returned bf16 block.

DMA (the roofline resource: ~25 MB/core at ~358 GB/s HBM): reads use
HWDGE (scalar/ACT ring), which sprays each transfer's per-partition
descriptors across all 16 SDMA engines by destination SBUF port
(~300 GB/s measured); all 4 input chunks are issued eagerly up front so
they stop contending with writes early. HWDGE does NOT spray
HBM-destined writes - all descriptors land on one engine
(~27 GB/s/ring, measured) - so most output writes go through SWDGE
(gpsimd): each DMA binds to one engine (~20 GB/s effective incl. the
per-512B 4-byte completion writes) and Tile's 8 DMASW sem lanes keep 8
in flight (~160 GB/s). On top of that, a few chunks are routed to the
otherwise-idle sync(SP)/scalar(ACT) HWDGE rings (~27 GB/s each).
Output is written g-major (out[g, b, 2D]) so one partition row is a
128 KB contiguous DRAM run: OUT_CHUNK=2 batches -> 127 descriptors of
4 KB per DMA.
"""

import numpy as np
import ml_dtypes

import concourse.bass as bass
import concourse.mybir as mybir
from concourse import bacc
from concourse.tile import TileContext
from concourse.bass_utils import run_bass_kernel_spmd

S, B, D = 128, 512, 512
G = S - 1
N_CORES = 8
BL = B // N_CORES  # 64 batches per core

IN_CHUNK = 16  # batches per input DMA  (16 KB per-partition descriptors)
OUT_CHUNK = 4  # batches per output DMA (8 KB per-partition descriptors)

_NC_CACHE = None


def _build_bass() -> bass.Bass:
    nc = bacc.Bacc()
    f32 = mybir.dt.float32
    bf16 = mybir.dt.bfloat16

    x_in = nc.declare_dram_parameter("x", [S, BL, D], bf16, isOutput=False)
    # tri[:, 0:128] = upper (incl diag)  tri[s,g] = 1 if s <= g  -> prefix sums
    # tri[:, 128:256] = strictly lower   tri[s,g] = 1 if s >  g  -> suffix sums
    t_in = nc.declare_dram_parameter("tri", [S, 2 * S], bf16, isOutput=False)
    # g-major per-core output: partition row g maps to a contiguous DRAM run,
    # host transposes (G, BL, 2D) -> (BL, G, 2D) while gathering.
    # Padded to S=128 rows: a 128-partition SBUF side lets the DGE spray the
    # transfer's descriptors across all 16 SDMA engines (8 partitions per
    # engine); a 127-partition AP falls off the swizzle path and the whole
    # DMA binds to ONE engine (~23 GB/s). Row 127 is garbage; host drops it.
    out = nc.declare_dram_parameter("out", [S, BL, 2 * D], bf16, isOutput=True)

    with TileContext(nc) as tc:
        with (
            tc.tile_pool(name="const", bufs=1) as cpool,
            tc.tile_pool(name="xin", bufs=4) as xpool,
            tc.tile_pool(name="outs", bufs=8) as opool,
            tc.tile_pool(name="psum", bufs=2, space="PSUM") as ppool,
        ):
            tri = cpool.tile([S, 2 * S], bf16)
            nc.sync.dma_start(out=tri[:], in_=t_in[:])
            ut = tri[:, 0:S]        # (128, 128) stationary, left sums
            lt = tri[:, S : 2 * S]  # (128, 128) stationary, right sums

            def per_pair(xt, ot, j, k, use_dve):
                """2 batches (j, j+1) of xt -> slots (k, k+1) of ot.

                One 4-bank PSUM tile takes all 4 matmuls (a matmul's output
                cannot exceed one 512-f32 PSUM bank per partition, so
                batches cannot share a wider matmul), then ONE 4-bank copy
                into ot. The copy engine alternates per ot tile (DVE for
                even chunks, ACT for odd): Tile tracks ot writes at tile
                granularity, so two engines writing disjoint slices of the
                same tile serialize on a false dependency - one tile, one
                engine keeps both copy engines fully parallel across chunks.
                """
                ps = ppool.tile([S, 4, D], f32)  # 4 adjacent banks
                for h, (b, tri_) in enumerate(
                    [(j, ut), (j, lt), (j + 1, ut), (j + 1, lt)]
                ):
                    nc.tensor.matmul(out=ps[:, h, :], lhsT=tri_, rhs=xt[:, b, :],
                                     start=True, stop=True)
                dst = ot[:, k : k + 2, :].rearrange(
                    "g c (h d) -> g (c h) d", h=2
                )
                if use_dve:
                    nc.vector.tensor_copy(out=dst, in_=ps[:, :, :])
                else:
                    nc.scalar.activation(
                        out=dst, in_=ps[:, :, :],
                        func=mybir.ActivationFunctionType.Copy,
                    )

            # issue all input loads eagerly (ACT HWDGE ring, sprays all 16
            # engines) so read traffic is done before writes ramp up.
            # (Tested worse: a 5-read split or routing the first read via
            # the sync ring - both delay the first write and/or drag the
            # read tail, costing 5-9us.)
            READS = [(0, 16), (16, 16), (32, 16), (48, 16)]
            xts = {}  # batch index of chunk start -> (tile, base batch)
            for r0, rn in READS:
                xt = xpool.tile([S, rn, D], bf16)
                nc.scalar.dma_start(out=xt[:], in_=x_in[:, r0 : r0 + rn, :])
                for b in range(r0, r0 + rn, OUT_CHUNK):
                    xts[b] = (xt, r0)

            n_chunks = BL // OUT_CHUNK
            for ci in range(n_chunks):
                o0 = ci * OUT_CHUNK
                xt, xbase = xts[o0]
                ot = opool.tile([S, OUT_CHUNK, 2 * D], bf16)
                for j in range(0, OUT_CHUNK, 2):
                    per_pair(xt, ot, o0 - xbase + j, j, ci % 2 == 0)
                nc.gpsimd.dma_start(
                    out=out[:, o0 : o0 + OUT_CHUNK, :], in_=ot[:, :, :],
                )
    nc.finalize()  # runs the Bacc pass pipeline (reg alloc, wait splitting)
    return nc


def _get_nc() -> bass.Bass:
    global _NC_CACHE
    if _NC_CACHE is None:
        _NC_CACHE = _build_bass()
    return _NC_CACHE


def _make_in_maps(x: np.ndarray, src_mask: np.ndarray) -> list[dict]:
    x = np.asarray(x, dtype=np.float32)
    src_mask = np.asarray(src_mask)
    assert x.shape == (S, B, D), x.shape
    assert src_mask.shape == (B, S), src_mask.shape

    valid = (~src_mask.astype(bool)).astype(np.float32).T  # (S, B)
    xm = (x * valid[:, :, None]).astype(ml_dtypes.bfloat16)
    tri = np.concatenate(
        [
            np.triu(np.ones((S, S), np.float32)),       # s <= g
            np.tril(np.ones((S, S), np.float32), -1),   # s >  g
        ],
        axis=1,
    ).astype(ml_dtypes.bfloat16)

    in_maps = []
    for i in range(N_CORES):
        sl = slice(i * BL, (i + 1) * BL)
        in_maps.append(
            {
                "x": np.ascontiguousarray(xm[:, sl, :]),
                "tri": tri,
            }
        )
    return in_maps


def _assemble(results: list[dict]) -> np.ndarray:
    full = np.empty((B, G, 2 * D), dtype=np.float32)
    for i in range(N_CORES):
        full[i * BL : (i + 1) * BL] = (
            results[i]["out"][:G].transpose(1, 0, 2).astype(np.float32)
        )
    return full


def kernel(x: np.ndarray, src_mask: np.ndarray) -> np.ndarray:
    in_maps = _make_in_maps(x, src_mask)
    res = run_bass_kernel_spmd(_get_nc(), in_maps, core_ids=list(range(N_CORES)))
    return _assemble(res.results)


# revision 30
# speedup vs baseline: 1.2131x; 1.0713x over previous
"""Trainium2 Bass kernel for nn_FragAttention (segment_reduce).

Reference computation (S=128, B=512, D=512, G=S-1=127):
    xb     = transpose(x, (1,0,2))            # (B, S, D)
    xm     = xb * (~src_mask)[:, :, None]     # zero padded tokens
    left   [b,g,d] = sum_{s<=g} xm[b,s,d]     # masked prefix sums
    right  [b,g,d] = sum_{s>g}  xm[b,s,d]
    out    = concat([left, right], axis=2)    # (B, G, 2D)

Strategy: pure data parallel over B across 8 cores (64 batches each).
The pad mask is folded into x on the host (exact: multiply by 0/1), and
x is cast to bf16 on the host — halves input HBM traffic; the 0/1
triangular weights are exact in bf16 so only x's mantissa truncation
matters (~2e-3 rel err, gate is 2e-2). Per batch the prefix/suffix sums
are computed on the TensorEngine as two matmuls against constant 0/1
triangular matrices (contraction over S=128 on partitions, f32 PSUM
accumulate), then PSUM->SBUF copies (DVE for even batches, ACT for odd)
cast the result to bf16. The host upcasts the returned bf16 block.

DMA (the roofline resource: ~25 MB/core at ~358 GB/s HBM): reads use
HWDGE (scalar/ACT ring), which sprays each transfer's per-partition
descriptors across all 16 SDMA engines by destination SBUF port
(~300 GB/s measured); all 4 input chunks are issued eagerly up front so
they stop contending with writes early. HWDGE does NOT spray
HBM-destined writes - all descriptors land on one engine
(~27 GB/s/ring, measured) - so most output writes go through SWDGE
(gpsimd): each DMA binds to one engine (~20 GB/s effective incl. the
per-512B 4-byte completion writes) and Tile's 8 DMASW sem lanes keep 8
in flight (~160 GB/s). On top of that, a few chunks are routed to the
otherwise-idle sync(SP)/scalar(ACT) HWDGE rings (~27 GB/s each).
Output is written g-major (out[g, b, 2D]) so one partition row is a
128 KB contiguous DRAM run: OUT_CHUNK=2 batches -> 127 descriptors of
4 KB per DMA.
"""

import numpy as np
import ml_dtypes

import concourse.bass as bass
import concourse.mybir as mybir
from concourse import bacc
from concourse.tile import TileContext
from concourse.bass_utils import run_bass_kernel_spmd

S, B, D = 128, 512, 512
G = S - 1
N_CORES = 8
BL = B // N_CORES  # 64 batches per core

IN_CHUNK = 16  # batches per input DMA  (16 KB per-partition descriptors)
OUT_CHUNK = 4  # batches per output DMA (8 KB per-partition descriptors)

_NC_CACHE = None


def _build_bass() -> bass.Bass:
    nc = bacc.Bacc()
    f32 = mybir.dt.float32
    bf16 = mybir.dt.bfloat16

    x_in = nc.declare_dram_parameter("x", [S, BL, D], bf16, isOutput=False)
    # tri[:, 0:128] = upper (incl diag)  tri[s,g] = 1 if s <= g  -> prefix sums
    # tri[:, 128:256] = strictly lower   tri[s,g] = 1 if s >  g  -> suffix sums
    t_in = nc.declare_dram_parameter("tri", [S, 2 * S], bf16, isOutput=False)
    # g-major per-core output: partition row g maps to a contiguous DRAM run,
    # host transposes (G, BL, 2D) -> (BL, G, 2D) while gathering.
    # Padded to S=128 rows: a 128-partition SBUF side lets the DGE spray the
    # transfer's descriptors across all 16 SDMA engines (8 partitions per
    # engine); a 127-partition AP falls off the swizzle path and the whole
    # DMA binds to ONE engine (~23 GB/s). Row 127 is garbage; host drops it.
    out = nc.declare_dram_parameter("out", [S, BL, 2 * D], bf16, isOutput=True)

    with TileContext(nc) as tc:
        with (
            tc.tile_pool(name="const", bufs=1) as cpool,
            tc.tile_pool(name="xin", bufs=4) as xpool,
            tc.tile_pool(name="outs", bufs=8) as opool,
            tc.tile_pool(name="psum", bufs=4, space="PSUM") as ppool,
        ):
            tri = cpool.tile([S, 2 * S], bf16)
            nc.sync.dma_start(out=tri[:], in_=t_in[:])
            ut = tri[:, 0:S]        # (128, 128) stationary, left sums
            lt = tri[:, S : 2 * S]  # (128, 128) stationary, right sums

            def per_pair(xt, ot, j, k, use_dve):
                """2 batches (j, j+1) of xt -> slots (k, k+1) of ot.

                One 4-bank PSUM tile takes all 4 matmuls (a matmul's output
                cannot exceed one 512-f32 PSUM bank per partition, so
                batches cannot share a wider matmul), then ONE 4-bank copy
                into ot. The copy engine alternates per ot tile (DVE for
                even chunks, ACT for odd): Tile tracks ot writes at tile
                granularity, so two engines writing disjoint slices of the
                same tile serialize on a false dependency - one tile, one
                engine keeps both copy engines fully parallel across chunks.
                """
                for c in range(2):
                    ps = ppool.tile([S, 2, D], f32)  # 2 adjacent banks
                    for h, tri_ in enumerate([ut, lt]):
                        nc.tensor.matmul(out=ps[:, h, :], lhsT=tri_,
                                         rhs=xt[:, j + c, :],
                                         start=True, stop=True)
                    dst = ot[:, k + c, :].rearrange("g (h d) -> g h d", h=2)
                    if use_dve:
                        nc.vector.tensor_copy(out=dst, in_=ps[:, :, :])
                    else:
                        nc.scalar.activation(
                            out=dst, in_=ps[:, :, :],
                            func=mybir.ActivationFunctionType.Copy,
                        )

            # issue all input loads eagerly (ACT HWDGE ring, sprays all 16
            # engines) so read traffic is done before writes ramp up.
            # (Tested worse: a 5-read split or routing the first read via
            # the sync ring - both delay the first write and/or drag the
            # read tail, costing 5-9us.)
            READS = [(0, 16), (16, 16), (32, 16), (48, 16)]
            xts = {}  # batch index of chunk start -> (tile, base batch)
            for r0, rn in READS:
                xt = xpool.tile([S, rn, D], bf16)
                nc.scalar.dma_start(out=xt[:], in_=x_in[:, r0 : r0 + rn, :])
                for b in range(r0, r0 + rn, OUT_CHUNK):
                    xts[b] = (xt, r0)

            n_chunks = BL // OUT_CHUNK
            for ci in range(n_chunks):
                o0 = ci * OUT_CHUNK
                xt, xbase = xts[o0]
                ot = opool.tile([S, OUT_CHUNK, 2 * D], bf16)
                for j in range(0, OUT_CHUNK, 2):
                    per_pair(xt, ot, o0 - xbase + j, j, ci % 2 == 0)
                nc.gpsimd.dma_start(
                    out=out[:, o0 : o0 + OUT_CHUNK, :], in_=ot[:, :, :],
                )
    nc.finalize()  # runs the Bacc pass pipeline (reg alloc, wait splitting)
    return nc


def _get_nc() -> bass.Bass:
    global _NC_CACHE
    if _NC_CACHE is None:
        _NC_CACHE = _build_bass()
    return _NC_CACHE


def _make_in_maps(x: np.ndarray, src_mask: np.ndarray) -> list[dict]:
    x = np.asarray(x, dtype=np.float32)
    src_mask = np.asarray(src_mask)
    assert x.shape == (S, B, D), x.shape
    assert src_mask.shape == (B, S), src_mask.shape

    valid = (~src_mask.astype(bool)).astype(np.float32).T  # (S, B)
    xm = (x * valid[:, :, None]).astype(ml_dtypes.bfloat16)
    tri = np.concatenate(
        [
            np.triu(np.ones((S, S), np.float32)),       # s <= g
            np.tril(np.ones((S, S), np.float32), -1),   # s >  g
        ],
        axis=1,
    ).astype(ml_dtypes.bfloat16)

    in_maps = []
    for i in range(N_CORES):
        sl = slice(i * BL, (i + 1) * BL)
        in_maps.append(
            {
                "x": np.ascontiguousarray(xm[:, sl, :]),
                "tri": tri,
            }
        )
    return in_maps


def _assemble(results: list[dict]) -> np.ndarray:
    full = np.empty((B, G, 2 * D), dtype=np.float32)
    for i in range(N_CORES):
        full[i * BL : (i + 1) * BL] = (
            results[i]["out"][:G].transpose(1, 0, 2).astype(np.float32)
        )
    return full


def kernel(x: np.ndarray, src_mask: np.ndarray) -> np.ndarray:
    in_maps = _make_in_maps(x, src_mask)
    res = run_bass_kernel_spmd(_get_nc(), in_maps, core_ids=list(range(N_CORES)))
    return _assemble(res.results)
